# revision 3
# baseline (speedup 1.0000x reference)
"""Distributed HGNN+ convolution for 8 Trainium2 NeuronCores (Bass/Tile).

Math (dense hypergraph incidence H [N_V, N_E], features X [N_V, C]):
    Xt  = X @ W.T + b                    # theta
    Xe  = (H.T @ Xt) * 1/colsum(H)       # V2E mean aggregation
    Xv  = (H @ Xe)   * 1/rowsum(H)       # E2V mean aggregation
    out = relu(Xv)

Distribution: vertex rows sharded across 8 cores; per-core partial V2E,
one bf16 AllReduce of the edge features (chunked in 2 so the first chunk
overlaps the V2E tail and the second overlaps the E2V head), then a fully
row-parallel E2V.

Kernel structure (the key difference vs the naive formulation): both big
GEMMs run with the *small* feature operand stationary in the PE array and
H / H.T as the 512-column moving operand:
    V2E:  psum[c128, e512] += Xt[v,c-tile].T @ H[v, e-slice]
    E2V:  psum[c128, v512] += Xe[e,c-tile].T @ H.T[e, v-slice]
This amortizes each LDWEIGHTS over 1024 streamed columns (the baseline
H-panel-stationary form paid one 128x128 weight load per 257-column
stream, ~460us of issued LDWEIGHTS).  The [c, x] outputs are flipped with
PE transposes (cheap: ~6% of stream time).

Degree vectors ride on idle engines: rowsum(H) = free-dim reduces of the
streamed H tiles (DVE), colsum(H) = gpsimd/DVE tile-tree-add + PE
transpose + free-dim reduce, then appended as the 513th column of the
AllReduce payload so the global edge degrees come back with Xe.
"""

import contextlib

import numpy as np
import ml_dtypes

BF16 = ml_dtypes.bfloat16

# Problem shape (hardcoded per contract).
N_V, N_E, CH, NCORES = 16384, 8192, 512, 8


def _full_cfg():
    return dict(n_v=N_V, n_e=N_E, ch=CH, ncores=NCORES)


def build_graph(tc, io, cfg):
    """Emit the Tile IR. io: dict of DRAM APs: hsp, htp, xta, wtb, out."""
    from concourse import mybir
    from concourse.masks import make_identity

    nc = tc.nc
    f32 = mybir.dt.float32
    bf16 = mybir.dt.bfloat16
    Relu = mybir.ActivationFunctionType.Relu
    AX = mybir.AxisListType.X
    ADD = mybir.AluOpType.add

    n_v, n_e, ch, ncores = cfg["n_v"], cfg["n_e"], cfg["ch"], cfg["ncores"]
    VS = n_v // ncores          # vertices per core (2048)
    KV = VS // 128              # vertex 128-tiles per core (16)
    EM = n_e // 128             # edge 128-tiles (64)
    NEG = n_e // 1024           # V2E e-groups of 1024 (8)
    NCH = 2                     # AllReduce chunks
    EGC = NEG // NCH            # e-groups per chunk (4)
    KEC = EM // NCH             # e-tiles per chunk (32)
    CT = ch // 128              # channel 128-tiles (4)
    CKT = ch // 128 + 1         # theta contraction tiles (X.T rows + ones row)
    SW = ch + 1                 # AR row width: channels + colsum column
    rg = [list(range(ncores))]

    hsp, htp, xta, wtb, out = io["hsp"], io["htp"], io["xta"], io["wtb"], io["out"]

    with contextlib.ExitStack() as ctx:
        pers = ctx.enter_context(tc.tile_pool(name="pers", bufs=1))
        dram = ctx.enter_context(tc.tile_pool(name="dram", bufs=1, space="DRAM"))

        ident_bf = pers.tile([128, 128], bf16)
        make_identity(nc, ident_bf)
        ident_f32 = pers.tile([128, 128], f32)
        make_identity(nc, ident_f32)

        # persistent across phases
        cs_cols = pers.tile([128, EM], f32)          # local colsum, [e-lane, ke]
        rs_parts = pers.tile([128, KV * NEG], f32)   # rowsum partials [v-lane, v*NEG+eg]
        rs_red = pers.tile([128, KV], f32)
        recip_rs = pers.tile([128, KV], f32)
        xe = [
            pers.tile([128, SW], bf16, tag="xe", bufs=EM, name=f"xe{ke}")
            for ke in range(EM)
        ]

        arin = [
            dram.tile([KEC * 128, SW], bf16, name=f"arin{c}", tag=f"arin{c}")
            for c in range(NCH)
        ]
        arout = [
            dram.tile([KEC * 128, SW], bf16, name=f"arout{c}", tag=f"arout{c}",
                      addr_space="Shared")
            for c in range(NCH)
        ]

        with contextlib.ExitStack() as actx:
            psumA = actx.enter_context(
                tc.tile_pool(name="psumA", bufs=1, space="PSUM"))
            apool = actx.enter_context(tc.tile_pool(name="apool", bufs=1))

            # ---- theta: Xt[v,c] tiles, bias via ones-row rank-1 update.
            with tc.tile_pool(name="tpool", bufs=1) as tpool:
                xta_sb = tpool.tile([128, CKT * VS], bf16)
                nc.sync.dma_start(
                    xta_sb.rearrange("p (k f) -> p k f", k=CKT),
                    xta.rearrange("(k p) f -> p k f", p=128),
                )
                wtb_sb = tpool.tile([128, CKT * ch], bf16)
                nc.sync.dma_start(
                    wtb_sb.rearrange("p (k f) -> p k f", k=CKT),
                    wtb.rearrange("(k p) f -> p k f", p=128),
                )

                # prefetch first two H e-groups (scalar-engine DMA queue)
                h_tiles = {}

                def fetch_h(eg):
                    for v in range(KV):
                        t = apool.tile([128, 1024], bf16, tag=f"h{v}", bufs=2,
                                       name=f"h{v}_{eg}")
                        nc.scalar.dma_start(t, hsp[v][:, eg * 1024:(eg + 1) * 1024])
                        h_tiles[(eg, v)] = t

                fetch_h(0)
                fetch_h(1)

                xt = []
                for v in range(KV):
                    ps = psumA.tile([128, ch], f32, tag="tr", bufs=2, name="ps_th")
                    for kt in range(CKT):
                        nc.tensor.matmul(
                            ps,
                            lhsT=xta_sb[:, kt * VS + v * 128: kt * VS + (v + 1) * 128],
                            rhs=wtb_sb[:, kt * ch:(kt + 1) * ch],
                            start=(kt == 0),
                            stop=(kt == CKT - 1),
                        )
                    xt_v = apool.tile([128, ch], bf16, tag="xt", bufs=KV,
                                      name=f"xt{v}")
                    nc.scalar.copy(xt_v, ps)
                    xt.append(xt_v)

            # ---- V2E + colsum/rowsum + transposes + chunked AllReduce.
            stage = {}
            acc = {}

            def emit_transposes(eg):
                """Transpose e-group eg's [c, e] output into [e, c] AR rows;
                finish its colsum; DMA to arin; fire the AR at chunk ends."""
                # colsum: transpose acc (f32) and reduce over the 128 v-lanes
                for kl in range(8):
                    ke = eg * 8 + kl
                    cst = psumA.tile([128, 128], f32, tag="cs_tr", bufs=2,
                                     name="cst")
                    nc.tensor.transpose(cst, acc[eg][:, kl * 128:(kl + 1) * 128],
                                        ident_f32)
                    nc.vector.tensor_reduce(cs_cols[:, ke:ke + 1], cst, AX, ADD)
                # data transposes + AR row assembly
                for kl in range(8):
                    ke = eg * 8 + kl
                    trp = psumA.tile([128, ch], bf16, tag="tr", bufs=2, name="trp")
                    for c in range(CT):
                        nc.tensor.transpose(
                            trp[:, c * 128:(c + 1) * 128],
                            stage[(eg, c)][:, kl * 128:(kl + 1) * 128],
                            ident_bf,
                        )
                    ar_t = apool.tile([128, SW], bf16, tag="ar", bufs=3,
                                      name="ar_t")
                    nc.scalar.copy(ar_t[:, 0:ch], trp)
                    nc.vector.tensor_copy(ar_t[:, ch:SW], cs_cols[:, ke:ke + 1])
                    chk, j = divmod(ke, KEC)
                    nc.sync.dma_start(arin[chk][j * 128:(j + 1) * 128, :], ar_t)
                if eg % EGC == EGC - 1:
                    chk = eg // EGC
                    nc.gpsimd.collective_compute(
                        "AllReduce",
                        ADD,
                        replica_groups=rg,
                        ins=[arin[chk].opt()],
                        outs=[arout[chk].opt()],
                    )

            for eg in range(NEG):
                for c in range(CT):
                    grp = psumA.tile([128, 1024], f32, tag="grp", bufs=2,
                                     name="grp")
                    for v in range(KV):
                        lhsT = xt[v][:, c * 128:(c + 1) * 128]
                        h = h_tiles[(eg, v)]
                        nc.tensor.matmul(grp[:, 0:512], lhsT=lhsT, rhs=h[:, 0:512],
                                         start=(v == 0), stop=(v == KV - 1))
                        nc.tensor.matmul(grp[:, 512:1024], lhsT=lhsT,
                                         rhs=h[:, 512:1024],
                                         start=(v == 0), stop=(v == KV - 1))
                    st = apool.tile([128, 1024], bf16, tag="stage", bufs=8,
                                    name=f"st{c}")
                    nc.scalar.copy(st[:, 0:512], grp[:, 0:512])
                    nc.scalar.copy(st[:, 512:1024], grp[:, 512:1024])
                    stage[(eg, c)] = st

                # colsum partial: tree-add the 16 H tiles (gpsimd leaves,
                # DVE internal adds), f32 accumulator
                g = apool.tile([128, 1024], f32, tag="acc", bufs=2, name="acc")
                t0 = apool.tile([128, 1024], f32, tag="tmp", bufs=2, name="tmp")
                nc.gpsimd.tensor_add(g, h_tiles[(eg, 0)], h_tiles[(eg, 1)])
                for k in range(2, KV, 2):
                    tk = apool.tile([128, 1024], f32, tag="tmp", bufs=2,
                                    name="tmp")
                    nc.gpsimd.tensor_add(tk, h_tiles[(eg, k)], h_tiles[(eg, k + 1)])
                    nc.vector.tensor_add(g, g, tk)
                acc[eg] = g

                # rowsum partials: free-dim reduce of each H tile (DVE)
                for v in range(KV):
                    nc.vector.tensor_reduce(
                        rs_parts[:, v * NEG + eg: v * NEG + eg + 1],
                        h_tiles[(eg, v)], AX, ADD)

                if eg + 2 < NEG:
                    fetch_h(eg + 2)
                if eg > 0:
                    emit_transposes(eg - 1)
            emit_transposes(NEG - 1)

            # rowsum finalize (no AR dependency)
            nc.vector.tensor_reduce(
                rs_red, rs_parts.rearrange("p (v e) -> p v e", v=KV), AX, ADD)
            nc.vector.reciprocal(recip_rs, rs_red)

            # ---- post-AR: load reduced Xe rows, scale by 1/colsum.
            for ke in range(EM):
                chk, j = divmod(ke, KEC)
                nc.sync.dma_start(xe[ke], arout[chk][j * 128:(j + 1) * 128, :])
                r = apool.tile([128, 1], f32, tag="rxe", bufs=4, name="rxe")
                nc.vector.reciprocal(r, xe[ke][:, ch:SW])
                nc.vector.tensor_scalar_mul(xe[ke][:, 0:ch], xe[ke][:, 0:ch], r)

        # ---- E2V: psum[c, v] += Xe[e, c-tile].T @ H.T[e, v-slice]
        with contextlib.ExitStack() as bctx:
            bpool = bctx.enter_context(tc.tile_pool(name="bpool", bufs=1))

            xvT = [
                bpool.tile([128, VS], bf16, tag="xvT", bufs=CT, name=f"xvT{c}")
                for c in range(CT)
            ]
            with tc.tile_pool(name="psumB", bufs=1, space="PSUM") as psumB:
                for vh in range(2):
                    g = psumB.tile([128, 4096], f32, tag="grpB", bufs=1,
                                   name="grpB")
                    for ke in range(EM):
                        ht = bpool.tile([128, 1024], bf16, tag="ht", bufs=8,
                                        name="ht")
                        nc.scalar.dma_start(
                            ht, htp[ke][:, vh * 1024:(vh + 1) * 1024])
                        for c in range(CT):
                            lhsT = xe[ke][:, c * 128:(c + 1) * 128]
                            nc.tensor.matmul(
                                g[:, (c * 2) * 512:(c * 2 + 1) * 512],
                                lhsT=lhsT, rhs=ht[:, 0:512],
                                start=(ke == 0), stop=(ke == EM - 1))
                            nc.tensor.matmul(
                                g[:, (c * 2 + 1) * 512:(c * 2 + 2) * 512],
                                lhsT=lhsT, rhs=ht[:, 512:1024],
                                start=(ke == 0), stop=(ke == EM - 1))
                    for c in range(CT):
                        for vc in range(2):
                            nc.scalar.copy(
                                xvT[c][:, vh * 1024 + vc * 512: vh * 1024 + (vc + 1) * 512],
                                g[:, (c * 2 + vc) * 512:(c * 2 + vc + 1) * 512])

            # output transposes + rowscale + ReLU + store
            with tc.tile_pool(name="psumC", bufs=1, space="PSUM") as psumC:
                for kt in range(KV):
                    trb = psumC.tile([128, ch], bf16, tag="trB", bufs=4,
                                     name="trb")
                    for c in range(CT):
                        nc.tensor.transpose(
                            trb[:, c * 128:(c + 1) * 128],
                            xvT[c][:, kt * 128:(kt + 1) * 128], ident_bf)
                    o = bpool.tile([128, ch], f32, tag="ost", bufs=3, name="o")
                    nc.scalar.activation(o, trb, Relu,
                                         scale=recip_rs[:, kt:kt + 1])
                    nc.sync.dma_start(out[kt * 128:(kt + 1) * 128, :], o)


def pack_inputs(X, H, W, b, cfg):
    """Host-side shard/cast/pack. Returns one input map per core."""
    from concurrent.futures import ThreadPoolExecutor

    n_v, n_e, ch, ncores = cfg["n_v"], cfg["n_e"], cfg["ch"], cfg["ncores"]
    VS = n_v // ncores
    KV = VS // 128
    EM = n_e // 128

    wtb = np.vstack(
        [
            np.ascontiguousarray(W.T).astype(np.float32),
            b[None, :].astype(np.float32),
            np.zeros((127, ch), np.float32),
        ]
    ).astype(BF16)

    H_bf = H.astype(BF16)

    def pack_core(c):
        Hc = H_bf[c * VS: (c + 1) * VS]
        # hsp[kt, p, e] = Hc[kt*128+p, e]   (V2E moving operand, [v, e])
        hsp = np.ascontiguousarray(Hc.reshape(KV, 128, n_e))
        # htp[ke, p, v] = Hc[v, ke*128+p]   (E2V moving operand, [e, v])
        htp = np.ascontiguousarray(Hc.T.reshape(EM, 128, VS))
        Xc = X[c * VS: (c + 1) * VS]
        xta = np.vstack(
            [
                np.ascontiguousarray(Xc.T),
                np.ones((1, VS), np.float32),
                np.zeros((127, VS), np.float32),
            ]
        ).astype(BF16)
        return dict(hsp=hsp, htp=htp, xta=xta, wtb=wtb)

    with ThreadPoolExecutor(max_workers=ncores) as ex:
        return list(ex.map(pack_core, range(ncores)))


_cache = {}


def _build_compiled(cfg, reps=1):
    key = (tuple(sorted(cfg.items())), reps)
    if key in _cache:
        return _cache[key]
    from concourse import bacc, mybir, tile

    n_v, n_e, ch, ncores = cfg["n_v"], cfg["n_e"], cfg["ch"], cfg["ncores"]
    VS = n_v // ncores
    KV = VS // 128
    EM = n_e // 128

    nc = bacc.Bacc("TRN2", target_bir_lowering=False, debug=False,
                   num_devices=ncores)
    io = {
        "hsp": nc.dram_tensor("hsp", [KV, 128, n_e], mybir.dt.bfloat16,
                              kind="ExternalInput").ap(),
        "htp": nc.dram_tensor("htp", [EM, 128, VS], mybir.dt.bfloat16,
                              kind="ExternalInput").ap(),
        "xta": nc.dram_tensor("xta", [ch + 128, VS], mybir.dt.bfloat16,
                              kind="ExternalInput").ap(),
        "wtb": nc.dram_tensor("wtb", [ch + 128, ch], mybir.dt.bfloat16,
                              kind="ExternalInput").ap(),
        "out": nc.dram_tensor("out", [VS, ch], mybir.dt.float32,
                              kind="ExternalOutput").ap(),
    }
    with tile.TileContext(nc) as tc:
        for _ in range(reps):
            build_graph(tc, io, cfg)
    nc.compile()
    _cache[key] = nc
    return nc


def kernel(X, H, W, b, _trace=False, _cfg=None):
    from concourse.bass_utils import run_bass_kernel_spmd

    cfg = _cfg or _full_cfg()
    X = np.asarray(X, dtype=np.float32)
    H = np.asarray(H, dtype=np.float32)
    W = np.asarray(W, dtype=np.float32)
    b = np.asarray(b, dtype=np.float32)

    nc = _build_compiled(cfg)
    in_maps = pack_inputs(X, H, W, b, cfg)
    res = run_bass_kernel_spmd(
        nc, in_maps, core_ids=list(range(cfg["ncores"])), trace=_trace
    )
    kernel.last_result = res
    return np.concatenate([r["out"] for r in res.results], axis=0)


kernel.last_result = None


# revision 7
# speedup vs baseline: 1.0280x; 1.0280x over previous
"""Distributed HGNN+ convolution for 8 Trainium2 NeuronCores (Bass/Tile).

Math (dense hypergraph incidence H [N_V, N_E], features X [N_V, C]):
    Xt  = X @ W.T + b                    # theta
    Xe  = (H.T @ Xt) * 1/colsum(H)       # V2E mean aggregation
    Xv  = (H @ Xe)   * 1/rowsum(H)       # E2V mean aggregation
    out = relu(Xv)

Distribution: vertex rows sharded across 8 cores; per-core partial V2E,
a bf16 AllReduce of the edge features chunked in 4 (each chunk fired as
soon as its edge range is computed, so all but the last chunk's latency
hides under the remaining V2E and the E2V front), then a fully
row-parallel E2V.

Kernel structure: both big GEMMs run with the *small* feature operand
stationary in the PE array and H / H.T as the 512-column moving operand:
    V2E:  psum[c128, e512] += Xt[v,c-tile].T @ H[v, e-slice]
    E2V:  psum[c128, v512] += Xe[e,c-tile].T @ H.T[e, v-slice]
(LDWEIGHTS fully overlaps the streams at this shape; the measured PE
cadence is ~264ns per 512-column matmul.)  The [c, x] outputs are flipped
back with PE transposes (~104ns each).

Degree vectors ride on otherwise-idle engine slack: rowsum(H) = free-dim
reduces of the streamed H tiles, colsum(H) = bf16 tile-tree-add + PE
transpose + free-dim reduce, appended as the 513th AllReduce column.
GpSimd runs nothing but the collectives so an AllReduce issue can never
queue behind slow elementwise work.
"""

import contextlib

import numpy as np
import ml_dtypes

BF16 = ml_dtypes.bfloat16

# Problem shape (hardcoded per contract).
N_V, N_E, CH, NCORES = 16384, 8192, 512, 8


def _full_cfg():
    return dict(n_v=N_V, n_e=N_E, ch=CH, ncores=NCORES, nch=4)


def build_graph(tc, io, cfg):
    """Emit the Tile IR. io: dict of DRAM APs: hsp, htp, xta, wtb, out."""
    from concourse import mybir
    from concourse.masks import make_identity

    nc = tc.nc
    f32 = mybir.dt.float32
    bf16 = mybir.dt.bfloat16
    Relu = mybir.ActivationFunctionType.Relu
    AX = mybir.AxisListType.X
    ADD = mybir.AluOpType.add

    n_v, n_e, ch, ncores = cfg["n_v"], cfg["n_e"], cfg["ch"], cfg["ncores"]
    NCH = cfg["nch"]            # AllReduce chunks
    VS = n_v // ncores          # vertices per core (2048)
    KV = VS // 128              # vertex 128-tiles per core (16)
    EM = n_e // 128             # edge 128-tiles (64)
    NEG = n_e // 1024           # V2E e-groups of 1024 (8)
    EGC = NEG // NCH            # e-groups per AR chunk
    KEC = EM // NCH             # e-tiles per AR chunk
    CT = ch // 128              # channel 128-tiles (4)
    CKT = ch // 128 + 1         # theta contraction tiles (X.T rows + ones row)
    SW = ch + 1                 # AR row width: channels + colsum column
    SWP = 528                   # padded row pitch in SBUF staging
    rg = [list(range(ncores))]

    hsp, htp, xta, wtb, out = io["hsp"], io["htp"], io["xta"], io["wtb"], io["out"]

    with contextlib.ExitStack() as ctx:
        pers = ctx.enter_context(tc.tile_pool(name="pers", bufs=1))
        dram = ctx.enter_context(tc.tile_pool(name="dram", bufs=1, space="DRAM"))

        ident_bf = pers.tile([128, 128], bf16)
        make_identity(nc, ident_bf)

        cs_cols = pers.tile([128, EM], f32)          # local colsum, [e-lane, ke]
        rs_parts = pers.tile([128, KV * NEG], f32)   # rowsum partials
        rs_red = pers.tile([128, KV], f32)
        recip_rs = pers.tile([128, KV], f32)
        xe_all = pers.tile([128, EM * SWP], bf16)    # reduced Xe rows [e, c]

        arin = [
            dram.tile([KEC * 128, SW], bf16, name=f"arin{c}", tag=f"arin{c}")
            for c in range(NCH)
        ]
        arout = [
            dram.tile([KEC * 128, SW], bf16, name=f"arout{c}", tag=f"arout{c}",
                      addr_space="Shared")
            for c in range(NCH)
        ]

        with contextlib.ExitStack() as actx:
            psumA = actx.enter_context(
                tc.tile_pool(name="psumA", bufs=1, space="PSUM"))
            apool = actx.enter_context(tc.tile_pool(name="apool", bufs=1))

            h_all = {}

            def fetch_h(eg):
                t = apool.tile([128, KV * 1024], bf16, tag="h", bufs=2,
                               name=f"h{eg}")
                nc.scalar.dma_start(
                    t.rearrange("p (v e) -> p v e", v=KV),
                    hsp.rearrange("v p e -> p v e")[:, :, eg * 1024:(eg + 1) * 1024],
                )
                h_all[eg] = t

            # ---- theta: Xt[v,c] tiles, bias via ones-row rank-1 update.
            with tc.tile_pool(name="tpool", bufs=1) as tpool:
                xta_sb = tpool.tile([128, CKT * VS], bf16)
                nc.sync.dma_start(
                    xta_sb.rearrange("p (k f) -> p k f", k=CKT),
                    xta.rearrange("(k p) f -> p k f", p=128),
                )
                wtb_sb = tpool.tile([128, CKT * ch], bf16)
                nc.sync.dma_start(
                    wtb_sb.rearrange("p (k f) -> p k f", k=CKT),
                    wtb.rearrange("(k p) f -> p k f", p=128),
                )
                fetch_h(0)
                fetch_h(1)

                xt = []
                for v in range(KV):
                    ps = psumA.tile([128, ch], f32, tag="tr", bufs=2, name="ps_th")
                    for kt in range(CKT):
                        nc.tensor.matmul(
                            ps,
                            lhsT=xta_sb[:, kt * VS + v * 128: kt * VS + (v + 1) * 128],
                            rhs=wtb_sb[:, kt * ch:(kt + 1) * ch],
                            start=(kt == 0),
                            stop=(kt == CKT - 1),
                        )
                    xt_v = apool.tile([128, ch], bf16, tag="xt", bufs=KV,
                                      name=f"xt{v}")
                    nc.scalar.copy(xt_v, ps)
                    xt.append(xt_v)

            # ---- V2E + degrees + transposes + chunked AllReduce.
            stage = {}
            acc = {}

            def emit_transposes(eg):
                """Flip e-group eg's [c, e] output into [e, c] AR rows, add
                its colsum column, DMA to arin, fire the AR at chunk ends."""
                # colsum: transpose bf16 acc, reduce over the 128 v-lanes
                for kl in range(8):
                    ke = eg * 8 + kl
                    cst = psumA.tile([128, 128], bf16, tag="cs_tr", bufs=2,
                                     name="cst")
                    nc.tensor.transpose(cst, acc[eg][:, kl * 128:(kl + 1) * 128],
                                        ident_bf)
                    nc.vector.tensor_reduce(cs_cols[:, ke:ke + 1], cst, AX, ADD)
                ar_t = apool.tile([128, 8 * SWP], bf16, tag="ar", bufs=1,
                                  name="ar_t")
                for kl in range(8):
                    ke = eg * 8 + kl
                    trp = psumA.tile([128, ch], bf16, tag="tr", bufs=2,
                                     name="trp")
                    for c in range(CT):
                        nc.tensor.transpose(
                            trp[:, c * 128:(c + 1) * 128],
                            stage[(eg, c)][:, kl * 128:(kl + 1) * 128],
                            ident_bf,
                        )
                    nc.scalar.copy(ar_t[:, kl * SWP:kl * SWP + ch], trp)
                    nc.vector.tensor_copy(ar_t[:, kl * SWP + ch:kl * SWP + SW],
                                          cs_cols[:, ke:ke + 1])
                chk, part = divmod(eg, EGC)
                nc.sync.dma_start(
                    arin[chk].rearrange(
                        "(k p) c -> p k c", p=128)[:, part * 8:(part + 1) * 8],
                    ar_t.rearrange("p (k c) -> p k c", k=8)[:, :, 0:SW],
                )
                if part == EGC - 1:
                    nc.gpsimd.collective_compute(
                        "AllReduce",
                        ADD,
                        replica_groups=rg,
                        ins=[arin[chk].opt()],
                        outs=[arout[chk].opt()],
                    )

            def consume_chunk(chk):
                """Post-AR: load reduced [e, c] rows into xe_all, scale by
                1/colsum (global edge degree came back as column 512)."""
                base = chk * KEC
                nc.sync.dma_start(
                    xe_all.rearrange(
                        "p (k c) -> p k c", k=EM)[:, base:base + KEC, 0:SW],
                    arout[chk].rearrange("(k p) c -> p k c", p=128),
                )
                r = apool.tile([128, KEC], f32, tag="rxe", bufs=2, name="rxe")
                nc.vector.reciprocal(
                    r,
                    xe_all.rearrange(
                        "p (k c) -> p k c", k=EM)[:, base:base + KEC, ch:SW],
                )
                for j in range(KEC):
                    ke = base + j
                    nc.vector.tensor_scalar_mul(
                        xe_all[:, ke * SWP:ke * SWP + ch],
                        xe_all[:, ke * SWP:ke * SWP + ch],
                        r[:, j:j + 1])

            for eg in range(NEG):
                for c in range(CT):
                    grp = psumA.tile([128, 1024], f32, tag="grp", bufs=2,
                                     name="grp")
                    for v in range(KV):
                        lhsT = xt[v][:, c * 128:(c + 1) * 128]
                        hv = h_all[eg][:, v * 1024:(v + 1) * 1024]
                        nc.tensor.matmul(grp[:, 0:512], lhsT=lhsT,
                                         rhs=hv[:, 0:512],
                                         start=(v == 0), stop=(v == KV - 1))
                        nc.tensor.matmul(grp[:, 512:1024], lhsT=lhsT,
                                         rhs=hv[:, 512:1024],
                                         start=(v == 0), stop=(v == KV - 1))
                    st = apool.tile([128, 1024], bf16, tag="stage", bufs=8,
                                    name=f"st{c}")
                    nc.scalar.copy(st[:, 0:512], grp[:, 0:512])
                    nc.scalar.copy(st[:, 512:1024], grp[:, 512:1024])
                    stage[(eg, c)] = st

                if eg + 2 < NEG:
                    fetch_h(eg + 2)

                # colsum partial: bf16 tree-add of the 16 H tiles (DVE)
                hv = lambda v: h_all[eg][:, v * 1024:(v + 1) * 1024]  # noqa: E731
                with nc.allow_low_precision("colsum tree in bf16"):
                    g = apool.tile([128, 1024], bf16, tag="acc", bufs=2,
                                   name="acc")
                    nc.vector.tensor_add(g, hv(0), hv(1))
                    for k in range(2, KV, 2):
                        tk = apool.tile([128, 1024], bf16, tag="tmp", bufs=2,
                                        name="tmp")
                        nc.vector.tensor_add(tk, hv(k), hv(k + 1))
                        nc.vector.tensor_add(g, g, tk)
                acc[eg] = g

                # rowsum partials: free-dim reduce of each H tile (DVE)
                for v in range(KV):
                    nc.vector.tensor_reduce(
                        rs_parts[:, v * NEG + eg: v * NEG + eg + 1],
                        hv(v), AX, ADD)

                if eg > 0:
                    emit_transposes(eg - 1)
                # consume AR chunks once they are safely complete
                if eg == 4:
                    consume_chunk(0)
                elif eg == 6:
                    consume_chunk(1)
                elif eg == 7:
                    consume_chunk(2)
            emit_transposes(NEG - 1)
            consume_chunk(3)

            # rowsum finalize (no AR dependency)
            nc.vector.tensor_reduce(
                rs_red, rs_parts.rearrange("p (v e) -> p v e", v=KV), AX, ADD)
            nc.vector.reciprocal(recip_rs, rs_red)

        # ---- E2V: psum[c, v] += Xe[e, c-tile].T @ H.T[e, v-slice]
        with contextlib.ExitStack() as bctx:
            bpool = bctx.enter_context(tc.tile_pool(name="bpool", bufs=1))

            xvT = [
                bpool.tile([128, VS], bf16, tag="xvT", bufs=CT, name=f"xvT{c}")
                for c in range(CT)
            ]
            with tc.tile_pool(name="psumB", bufs=1, space="PSUM") as psumB:
                for vh in range(2):
                    g = psumB.tile([128, 4096], f32, tag="grpB", bufs=1,
                                   name="grpB")
                    for kg in range(EM // 8):
                        ht = bpool.tile([128, 8 * 1024], bf16, tag="ht",
                                        bufs=2, name="ht")
                        nc.scalar.dma_start(
                            ht.rearrange("p (k v) -> p k v", k=8),
                            htp.rearrange(
                                "k p v -> p k v")[:, kg * 8:(kg + 1) * 8,
                                                  vh * 1024:(vh + 1) * 1024],
                        )
                        for kl in range(8):
                            ke = kg * 8 + kl
                            for c in range(CT):
                                lhsT = xe_all[:, ke * SWP + c * 128:
                                              ke * SWP + (c + 1) * 128]
                                nc.tensor.matmul(
                                    g[:, (c * 2) * 512:(c * 2 + 1) * 512],
                                    lhsT=lhsT,
                                    rhs=ht[:, kl * 1024:kl * 1024 + 512],
                                    start=(ke == 0), stop=(ke == EM - 1))
                                nc.tensor.matmul(
                                    g[:, (c * 2 + 1) * 512:(c * 2 + 2) * 512],
                                    lhsT=lhsT,
                                    rhs=ht[:, kl * 1024 + 512:(kl + 1) * 1024],
                                    start=(ke == 0), stop=(ke == EM - 1))
                    for c in range(CT):
                        for vc in range(2):
                            nc.scalar.copy(
                                xvT[c][:, vh * 1024 + vc * 512:
                                       vh * 1024 + (vc + 1) * 512],
                                g[:, (c * 2 + vc) * 512:(c * 2 + vc + 1) * 512])

            # output transposes + rowscale + ReLU + store
            with tc.tile_pool(name="psumC", bufs=1, space="PSUM") as psumC:
                for kt in range(KV):
                    trb = psumC.tile([128, ch], bf16, tag="trB", bufs=4,
                                     name="trb")
                    for c in range(CT):
                        nc.tensor.transpose(
                            trb[:, c * 128:(c + 1) * 128],
                            xvT[c][:, kt * 128:(kt + 1) * 128], ident_bf)
                    o = bpool.tile([128, ch], f32, tag="ost", bufs=3, name="o")
                    nc.scalar.activation(o, trb, Relu,
                                         scale=recip_rs[:, kt:kt + 1])
                    nc.sync.dma_start(out[kt * 128:(kt + 1) * 128, :], o)


def pack_inputs(X, H, W, b, cfg):
    """Host-side shard/cast/pack. Returns one input map per core."""
    from concurrent.futures import ThreadPoolExecutor

    n_v, n_e, ch, ncores = cfg["n_v"], cfg["n_e"], cfg["ch"], cfg["ncores"]
    VS = n_v // ncores
    KV = VS // 128
    EM = n_e // 128

    wtb = np.vstack(
        [
            np.ascontiguousarray(W.T).astype(np.float32),
            b[None, :].astype(np.float32),
            np.zeros((127, ch), np.float32),
        ]
    ).astype(BF16)

    H_bf = H.astype(BF16)

    def pack_core(c):
        Hc = H_bf[c * VS: (c + 1) * VS]
        # hsp[kt, p, e] = Hc[kt*128+p, e]   (V2E moving operand, [v, e])
        hsp = np.ascontiguousarray(Hc.reshape(KV, 128, n_e))
        # htp[ke, p, v] = Hc[v, ke*128+p]   (E2V moving operand, [e, v])
        htp = np.ascontiguousarray(Hc.T.reshape(EM, 128, VS))
        Xc = X[c * VS: (c + 1) * VS]
        xta = np.vstack(
            [
                np.ascontiguousarray(Xc.T),
                np.ones((1, VS), np.float32),
                np.zeros((127, VS), np.float32),
            ]
        ).astype(BF16)
        return dict(hsp=hsp, htp=htp, xta=xta, wtb=wtb)

    with ThreadPoolExecutor(max_workers=ncores) as ex:
        return list(ex.map(pack_core, range(ncores)))


_cache = {}


def _build_compiled(cfg, reps=1):
    key = (tuple(sorted(cfg.items())), reps)
    if key in _cache:
        return _cache[key]
    from concourse import bacc, mybir, tile

    n_v, n_e, ch, ncores = cfg["n_v"], cfg["n_e"], cfg["ch"], cfg["ncores"]
    VS = n_v // ncores
    KV = VS // 128
    EM = n_e // 128

    nc = bacc.Bacc("TRN2", target_bir_lowering=False, debug=False,
                   num_devices=ncores)
    io = {
        "hsp": nc.dram_tensor("hsp", [KV, 128, n_e], mybir.dt.bfloat16,
                              kind="ExternalInput").ap(),
        "htp": nc.dram_tensor("htp", [EM, 128, VS], mybir.dt.bfloat16,
                              kind="ExternalInput").ap(),
        "xta": nc.dram_tensor("xta", [ch + 128, VS], mybir.dt.bfloat16,
                              kind="ExternalInput").ap(),
        "wtb": nc.dram_tensor("wtb", [ch + 128, ch], mybir.dt.bfloat16,
                              kind="ExternalInput").ap(),
        "out": nc.dram_tensor("out", [VS, ch], mybir.dt.float32,
                              kind="ExternalOutput").ap(),
    }
    with tile.TileContext(nc) as tc:
        for _ in range(reps):
            build_graph(tc, io, cfg)
    nc.compile()
    _cache[key] = nc
    return nc


def kernel(X, H, W, b, _trace=False, _cfg=None):
    from concourse.bass_utils import run_bass_kernel_spmd

    cfg = _cfg or _full_cfg()
    X = np.asarray(X, dtype=np.float32)
    H = np.asarray(H, dtype=np.float32)
    W = np.asarray(W, dtype=np.float32)
    b = np.asarray(b, dtype=np.float32)

    nc = _build_compiled(cfg)
    in_maps = pack_inputs(X, H, W, b, cfg)
    res = run_bass_kernel_spmd(
        nc, in_maps, core_ids=list(range(cfg["ncores"])), trace=_trace
    )
    kernel.last_result = res
    return np.concatenate([r["out"] for r in res.results], axis=0)


kernel.last_result = None


# revision 8
# speedup vs baseline: 1.0381x; 1.0098x over previous
"""Distributed HGNN+ convolution for 8 Trainium2 NeuronCores (Bass/Tile).

Math (dense hypergraph incidence H [N_V, N_E], features X [N_V, C]):
    Xt  = X @ W.T + b                    # theta
    Xe  = (H.T @ Xt) * 1/colsum(H)       # V2E mean aggregation
    Xv  = (H @ Xe)   * 1/rowsum(H)       # E2V mean aggregation
    out = relu(Xv)

Distribution: vertex rows sharded across 8 cores; per-core partial V2E,
a bf16 AllReduce of the edge features chunked in 4 (each chunk fired as
soon as its edge range is computed, so all but the last chunk's latency
hides under the remaining V2E and the E2V front), then a fully
row-parallel E2V.

Kernel structure: both big GEMMs run with the *small* feature operand
stationary in the PE array and H / H.T as the 512-column moving operand:
    V2E:  psum[c128, e512] += Xt[v,c-tile].T @ H[v, e-slice]
    E2V:  psum[c128, v512] += Xe[e,c-tile].T @ H.T[e, v-slice]
(LDWEIGHTS fully overlaps the streams at this shape; the measured PE
cadence is ~264ns per 512-column matmul.)  The [c, x] outputs are flipped
back with PE transposes (~104ns each).

Degree vectors ride on otherwise-idle engine slack: rowsum(H) = free-dim
reduces of the streamed H tiles, colsum(H) = bf16 tile-tree-add + PE
transpose + free-dim reduce, appended as the 513th AllReduce column.
GpSimd runs nothing but the collectives so an AllReduce issue can never
queue behind slow elementwise work.
"""

import contextlib

import numpy as np
import ml_dtypes

BF16 = ml_dtypes.bfloat16

# Problem shape (hardcoded per contract).
N_V, N_E, CH, NCORES = 16384, 8192, 512, 8


def _full_cfg():
    return dict(n_v=N_V, n_e=N_E, ch=CH, ncores=NCORES, nch=4)


def build_graph(tc, io, cfg):
    """Emit the Tile IR. io: dict of DRAM APs: hsp, htp, xta, wtb, out."""
    from concourse import mybir
    from concourse.masks import make_identity

    nc = tc.nc
    f32 = mybir.dt.float32
    bf16 = mybir.dt.bfloat16
    Relu = mybir.ActivationFunctionType.Relu
    AX = mybir.AxisListType.X
    ADD = mybir.AluOpType.add

    n_v, n_e, ch, ncores = cfg["n_v"], cfg["n_e"], cfg["ch"], cfg["ncores"]
    NCH = cfg["nch"]            # AllReduce chunks
    VS = n_v // ncores          # vertices per core (2048)
    KV = VS // 128              # vertex 128-tiles per core (16)
    EM = n_e // 128             # edge 128-tiles (64)
    NEG = n_e // 1024           # V2E e-groups of 1024 (8)
    EGC = NEG // NCH            # e-groups per AR chunk
    KEC = EM // NCH             # e-tiles per AR chunk
    CT = ch // 128              # channel 128-tiles (4)
    CKT = ch // 128 + 1         # theta contraction tiles (X.T rows + ones row)
    SW = ch + 1                 # AR row width: channels + colsum column
    SWP = 528                   # padded row pitch in SBUF staging
    rg = [list(range(ncores))]

    hsp, htp, xta, wtb, out = io["hsp"], io["htp"], io["xta"], io["wtb"], io["out"]

    with contextlib.ExitStack() as ctx:
        pers = ctx.enter_context(tc.tile_pool(name="pers", bufs=1))
        dram = ctx.enter_context(tc.tile_pool(name="dram", bufs=1, space="DRAM"))

        ident_bf = pers.tile([128, 128], bf16)
        make_identity(nc, ident_bf)

        cs_cols = pers.tile([128, EM], f32)          # local colsum, [e-lane, ke]
        rs_parts = pers.tile([128, KV * NEG], f32)   # rowsum partials
        rs_red = pers.tile([128, KV], f32)
        recip_rs = pers.tile([128, KV], f32)
        xe_all = pers.tile([128, EM * SWP], bf16)    # reduced Xe rows [e, c]

        arin = [
            dram.tile([KEC * 128, SW], bf16, name=f"arin{c}", tag=f"arin{c}")
            for c in range(NCH)
        ]
        arout = [
            dram.tile([KEC * 128, SW], bf16, name=f"arout{c}", tag=f"arout{c}",
                      addr_space="Shared")
            for c in range(NCH)
        ]

        with contextlib.ExitStack() as actx:
            psumA = actx.enter_context(
                tc.tile_pool(name="psumA", bufs=1, space="PSUM"))
            apool = actx.enter_context(tc.tile_pool(name="apool", bufs=1))

            h_all = {}

            def fetch_h(eg):
                t = apool.tile([128, KV * 1024], bf16, tag="h", bufs=2,
                               name=f"h{eg}")
                nc.scalar.dma_start(
                    t.rearrange("p (v e) -> p v e", v=KV),
                    hsp.rearrange("v p e -> p v e")[:, :, eg * 1024:(eg + 1) * 1024],
                )
                h_all[eg] = t

            # ---- theta: Xt[v,c] tiles, bias via ones-row rank-1 update.
            with tc.tile_pool(name="tpool", bufs=1) as tpool:
                xta_sb = tpool.tile([128, CKT * VS], bf16)
                nc.sync.dma_start(
                    xta_sb.rearrange("p (k f) -> p k f", k=CKT),
                    xta.rearrange("(k p) f -> p k f", p=128),
                )
                wtb_sb = tpool.tile([128, CKT * ch], bf16)
                nc.sync.dma_start(
                    wtb_sb.rearrange("p (k f) -> p k f", k=CKT),
                    wtb.rearrange("(k p) f -> p k f", p=128),
                )
                fetch_h(0)
                fetch_h(1)

                xt = []
                for v in range(KV):
                    ps = psumA.tile([128, ch], f32, tag="tr", bufs=2, name="ps_th")
                    for kt in range(CKT):
                        nc.tensor.matmul(
                            ps,
                            lhsT=xta_sb[:, kt * VS + v * 128: kt * VS + (v + 1) * 128],
                            rhs=wtb_sb[:, kt * ch:(kt + 1) * ch],
                            start=(kt == 0),
                            stop=(kt == CKT - 1),
                        )
                    xt_v = apool.tile([128, ch], bf16, tag="xt", bufs=KV,
                                      name=f"xt{v}")
                    nc.scalar.copy(xt_v, ps)
                    xt.append(xt_v)

            # ---- V2E + degrees + transposes + chunked AllReduce.
            stage = {}
            acc = {}

            def emit_transposes(eg):
                """Flip e-group eg's [c, e] output into [e, c] AR rows, add
                its colsum column, DMA to arin, fire the AR at chunk ends."""
                # colsum: transpose bf16 acc, reduce over the 128 v-lanes
                for kl in range(8):
                    ke = eg * 8 + kl
                    cst = psumA.tile([128, 128], bf16, tag="cs_tr", bufs=2,
                                     name="cst")
                    nc.tensor.transpose(cst, acc[eg][:, kl * 128:(kl + 1) * 128],
                                        ident_bf)
                    nc.vector.tensor_reduce(cs_cols[:, ke:ke + 1], cst, AX, ADD)
                ar_t = apool.tile([128, 8 * SWP], bf16, tag="ar", bufs=1,
                                  name="ar_t")
                for kl in range(8):
                    ke = eg * 8 + kl
                    trp = psumA.tile([128, ch], bf16, tag="tr", bufs=2,
                                     name="trp")
                    for c in range(CT):
                        nc.tensor.transpose(
                            trp[:, c * 128:(c + 1) * 128],
                            stage[(eg, c)][:, kl * 128:(kl + 1) * 128],
                            ident_bf,
                        )
                    nc.scalar.copy(ar_t[:, kl * SWP:kl * SWP + ch], trp)
                    nc.vector.tensor_copy(ar_t[:, kl * SWP + ch:kl * SWP + SW],
                                          cs_cols[:, ke:ke + 1])
                chk, part = divmod(eg, EGC)
                nc.scalar.dma_start(
                    arin[chk].rearrange(
                        "(k p) c -> p k c", p=128)[:, part * 8:(part + 1) * 8],
                    ar_t.rearrange("p (k c) -> p k c", k=8)[:, :, 0:SW],
                )
                if part == EGC - 1:
                    nc.gpsimd.collective_compute(
                        "AllReduce",
                        ADD,
                        replica_groups=rg,
                        ins=[arin[chk].opt()],
                        outs=[arout[chk].opt()],
                    )

            def consume_chunk(chk):
                """Post-AR: load reduced [e, c] rows into xe_all, scale by
                1/colsum (global edge degree came back as column 512)."""
                base = chk * KEC
                nc.sync.dma_start(
                    xe_all.rearrange(
                        "p (k c) -> p k c", k=EM)[:, base:base + KEC, 0:SW],
                    arout[chk].rearrange("(k p) c -> p k c", p=128),
                )
                r = apool.tile([128, KEC], f32, tag="rxe", bufs=2, name="rxe")
                nc.vector.reciprocal(
                    r,
                    xe_all.rearrange(
                        "p (k c) -> p k c", k=EM)[:, base:base + KEC, ch:SW],
                )
                for j in range(KEC):
                    ke = base + j
                    nc.vector.tensor_scalar_mul(
                        xe_all[:, ke * SWP:ke * SWP + ch],
                        xe_all[:, ke * SWP:ke * SWP + ch],
                        r[:, j:j + 1])

            for eg in range(NEG):
                for c in range(CT):
                    grp = psumA.tile([128, 1024], f32, tag="grp", bufs=2,
                                     name="grp")
                    for v in range(KV):
                        lhsT = xt[v][:, c * 128:(c + 1) * 128]
                        hv = h_all[eg][:, v * 1024:(v + 1) * 1024]
                        nc.tensor.matmul(grp[:, 0:512], lhsT=lhsT,
                                         rhs=hv[:, 0:512],
                                         start=(v == 0), stop=(v == KV - 1))
                        nc.tensor.matmul(grp[:, 512:1024], lhsT=lhsT,
                                         rhs=hv[:, 512:1024],
                                         start=(v == 0), stop=(v == KV - 1))
                    st = apool.tile([128, 1024], bf16, tag="stage", bufs=8,
                                    name=f"st{c}")
                    nc.scalar.copy(st[:, 0:512], grp[:, 0:512])
                    nc.scalar.copy(st[:, 512:1024], grp[:, 512:1024])
                    stage[(eg, c)] = st

                if eg + 2 < NEG:
                    fetch_h(eg + 2)

                # colsum partial: bf16 tree-add of the 16 H tiles (DVE)
                hv = lambda v: h_all[eg][:, v * 1024:(v + 1) * 1024]  # noqa: E731
                with nc.allow_low_precision("colsum tree in bf16"):
                    g = apool.tile([128, 1024], bf16, tag="acc", bufs=2,
                                   name="acc")
                    nc.gpsimd.tensor_add(g, hv(0), hv(1))
                    for k in range(2, KV, 2):
                        tk = apool.tile([128, 1024], bf16, tag="tmp", bufs=2,
                                        name="tmp")
                        nc.gpsimd.tensor_add(tk, hv(k), hv(k + 1))
                        nc.gpsimd.tensor_add(g, g, tk)
                acc[eg] = g

                # rowsum partials: free-dim reduce of each H tile (DVE)
                for v in range(KV):
                    nc.vector.tensor_reduce(
                        rs_parts[:, v * NEG + eg: v * NEG + eg + 1],
                        hv(v), AX, ADD)

                if eg > 0:
                    emit_transposes(eg - 1)
                # consume AR chunks once they are safely complete
                if eg == 5:
                    consume_chunk(0)
                elif eg == 7:
                    consume_chunk(1)
            emit_transposes(NEG - 1)

            # rowsum finalize (no AR dependency; before the chunk-3 wait)
            nc.vector.tensor_reduce(
                rs_red, rs_parts.rearrange("p (v e) -> p v e", v=KV), AX, ADD)
            nc.vector.reciprocal(recip_rs, rs_red)

            consume_chunk(2)
            consume_chunk(3)

        # ---- E2V: psum[c, v] += Xe[e, c-tile].T @ H.T[e, v-slice]
        with contextlib.ExitStack() as bctx:
            bpool = bctx.enter_context(tc.tile_pool(name="bpool", bufs=1))

            xvT = [
                bpool.tile([128, VS], bf16, tag="xvT", bufs=CT, name=f"xvT{c}")
                for c in range(CT)
            ]
            with tc.tile_pool(name="psumB", bufs=1, space="PSUM") as psumB:
                for vh in range(2):
                    g = psumB.tile([128, 4096], f32, tag="grpB", bufs=1,
                                   name="grpB")
                    for kg in range(EM // 8):
                        ht = bpool.tile([128, 8 * 1024], bf16, tag="ht",
                                        bufs=2, name="ht")
                        nc.scalar.dma_start(
                            ht.rearrange("p (k v) -> p k v", k=8),
                            htp.rearrange(
                                "k p v -> p k v")[:, kg * 8:(kg + 1) * 8,
                                                  vh * 1024:(vh + 1) * 1024],
                        )
                        for kl in range(8):
                            ke = kg * 8 + kl
                            for c in range(CT):
                                lhsT = xe_all[:, ke * SWP + c * 128:
                                              ke * SWP + (c + 1) * 128]
                                nc.tensor.matmul(
                                    g[:, (c * 2) * 512:(c * 2 + 1) * 512],
                                    lhsT=lhsT,
                                    rhs=ht[:, kl * 1024:kl * 1024 + 512],
                                    start=(ke == 0), stop=(ke == EM - 1))
                                nc.tensor.matmul(
                                    g[:, (c * 2 + 1) * 512:(c * 2 + 2) * 512],
                                    lhsT=lhsT,
                                    rhs=ht[:, kl * 1024 + 512:(kl + 1) * 1024],
                                    start=(ke == 0), stop=(ke == EM - 1))
                    for c in range(CT):
                        for vc in range(2):
                            nc.scalar.copy(
                                xvT[c][:, vh * 1024 + vc * 512:
                                       vh * 1024 + (vc + 1) * 512],
                                g[:, (c * 2 + vc) * 512:(c * 2 + vc + 1) * 512])

            # output transposes + rowscale + ReLU + store
            with tc.tile_pool(name="psumC", bufs=1, space="PSUM") as psumC:
                for kt in range(KV):
                    trb = psumC.tile([128, ch], bf16, tag="trB", bufs=4,
                                     name="trb")
                    for c in range(CT):
                        nc.tensor.transpose(
                            trb[:, c * 128:(c + 1) * 128],
                            xvT[c][:, kt * 128:(kt + 1) * 128], ident_bf)
                    o = bpool.tile([128, ch], f32, tag="ost", bufs=3, name="o")
                    nc.scalar.activation(o, trb, Relu,
                                         scale=recip_rs[:, kt:kt + 1])
                    nc.sync.dma_start(out[kt * 128:(kt + 1) * 128, :], o)


def pack_inputs(X, H, W, b, cfg):
    """Host-side shard/cast/pack. Returns one input map per core."""
    from concurrent.futures import ThreadPoolExecutor

    n_v, n_e, ch, ncores = cfg["n_v"], cfg["n_e"], cfg["ch"], cfg["ncores"]
    VS = n_v // ncores
    KV = VS // 128
    EM = n_e // 128

    wtb = np.vstack(
        [
            np.ascontiguousarray(W.T).astype(np.float32),
            b[None, :].astype(np.float32),
            np.zeros((127, ch), np.float32),
        ]
    ).astype(BF16)

    H_bf = H.astype(BF16)

    def pack_core(c):
        Hc = H_bf[c * VS: (c + 1) * VS]
        # hsp[kt, p, e] = Hc[kt*128+p, e]   (V2E moving operand, [v, e])
        hsp = np.ascontiguousarray(Hc.reshape(KV, 128, n_e))
        # htp[ke, p, v] = Hc[v, ke*128+p]   (E2V moving operand, [e, v])
        htp = np.ascontiguousarray(Hc.T.reshape(EM, 128, VS))
        Xc = X[c * VS: (c + 1) * VS]
        xta = np.vstack(
            [
                np.ascontiguousarray(Xc.T),
                np.ones((1, VS), np.float32),
                np.zeros((127, VS), np.float32),
            ]
        ).astype(BF16)
        return dict(hsp=hsp, htp=htp, xta=xta, wtb=wtb)

    with ThreadPoolExecutor(max_workers=ncores) as ex:
        return list(ex.map(pack_core, range(ncores)))


_cache = {}


def _build_compiled(cfg, reps=1):
    key = (tuple(sorted(cfg.items())), reps)
    if key in _cache:
        return _cache[key]
    from concourse import bacc, mybir, tile

    n_v, n_e, ch, ncores = cfg["n_v"], cfg["n_e"], cfg["ch"], cfg["ncores"]
    VS = n_v // ncores
    KV = VS // 128
    EM = n_e // 128

    nc = bacc.Bacc("TRN2", target_bir_lowering=False, debug=False,
                   num_devices=ncores)
    io = {
        "hsp": nc.dram_tensor("hsp", [KV, 128, n_e], mybir.dt.bfloat16,
                              kind="ExternalInput").ap(),
        "htp": nc.dram_tensor("htp", [EM, 128, VS], mybir.dt.bfloat16,
                              kind="ExternalInput").ap(),
        "xta": nc.dram_tensor("xta", [ch + 128, VS], mybir.dt.bfloat16,
                              kind="ExternalInput").ap(),
        "wtb": nc.dram_tensor("wtb", [ch + 128, ch], mybir.dt.bfloat16,
                              kind="ExternalInput").ap(),
        "out": nc.dram_tensor("out", [VS, ch], mybir.dt.float32,
                              kind="ExternalOutput").ap(),
    }
    with tile.TileContext(nc) as tc:
        for _ in range(reps):
            build_graph(tc, io, cfg)
    nc.compile()
    _cache[key] = nc
    return nc


def kernel(X, H, W, b, _trace=False, _cfg=None):
    from concourse.bass_utils import run_bass_kernel_spmd

    cfg = _cfg or _full_cfg()
    X = np.asarray(X, dtype=np.float32)
    H = np.asarray(H, dtype=np.float32)
    W = np.asarray(W, dtype=np.float32)
    b = np.asarray(b, dtype=np.float32)

    nc = _build_compiled(cfg)
    in_maps = pack_inputs(X, H, W, b, cfg)
    res = run_bass_kernel_spmd(
        nc, in_maps, core_ids=list(range(cfg["ncores"])), trace=_trace
    )
    kernel.last_result = res
    return np.concatenate([r["out"] for r in res.results], axis=0)


kernel.last_result = None
